# revision 28
# baseline (speedup 1.0000x reference)
"""Trainium2 Bass kernel for per-(batch,channel) circular autocorrelation:

    out = ifft2(|fft2(img - mean(img))|^2).real / (H*W)

Implemented as dense DFT matmuls on the TensorEngine (no FFT primitive on
TRN2). Key algebra:
  * mean subtraction == zeroing the DC bin of the power spectrum, so we
    transform the raw image and memset P[0,0] = 0.
  * real input => half spectrum (j = 0..128) suffices; mirror rows are
    folded into the inverse-transform constants with weight 2.
  * DFT matrix C = cos - i sin is symmetric, which lets every stage run as
    out = lhsT.T @ rhs with operands already in natural layout (no
    transposes anywhere).
  * the 129-row half spectrum is split as j=1..128 (dense 128-row block)
    plus a j=0 rank-1 fixup. The j=0 row only needs the cosine half of the
    inverse transform (sin(0)=0) and its y-weight is the constant 1/N^4,
    so the fixup costs two narrow matmuls.

Stages per 256x256 image (Co/Si are 256x256 cos/sin matrices):
  s1: [U|V] = d^T [CoH|SiH]              (fp32r matmuls, FD=258)
  s2: [G2|-G1] accumulated in PSUM: wide Si x [U|V] plus narrow +Co x V /
      -Co x U; the -G1 sign is absorbed by the square  (bf16)
  P:  Pt = G1^2 + G2^2 (bf16), Pt[0,0]=0
  s4a: Qr|Qi = Pt[:,1:129]^T [Co|Si] ; q0 = Pt[:,0]^T Co   (bf16)
  s4b: out[y,x] = sum_{j=1..128} w_j/N^4 (cos(2pi jy/N) Qr - sin(..) Qi)
       + q0[x]/N^4   for all 256 y rows (mirror weights baked in).

Sharding: pure data parallel over 8 cores, 64 images (8 batches x 8
channels) per core. All bf16 constants ship as ONE dram tensor (fewer
runtime buffers per call = lower per-call dispatch cost).
"""

import numpy as np

N = 256
J = N // 2 + 1  # 129
B, H, W, C = 64, 256, 256, 8
N_CORES = 8
IMGS_PER_CORE = (B // N_CORES) * C  # 64

# bf16 const blob layout: [256, 1024]
#   cols    0:512  rows 0:256 -> [Co | Si]
#   cols 512:1024  rows 0:128 -> [cw2 | sw2]    (j = 1..128 inverse weights)
CBLOB_COLS = 1024


def _make_consts():
    import ml_dtypes

    bf = ml_dtypes.bfloat16
    a = np.arange(N, dtype=np.float64)
    j = np.arange(J, dtype=np.float64)
    ang = 2.0 * np.pi / N

    CoH = np.cos(ang * np.outer(a, j))  # [256, 129]
    SiH = np.sin(ang * np.outer(a, j))
    cosih = np.concatenate([CoH, SiH], axis=1).astype(np.float32)  # [256, 258]

    Co = np.cos(ang * np.outer(a, a))
    Si = np.sin(ang * np.outer(a, a))

    s = 1.0 / float(N) ** 4
    jm = np.arange(1, 129, dtype=np.float64)  # j = 1..128
    w = np.full(128, 2.0)
    w[127] = 1.0  # j=128 self-conjugate row
    cw2 = s * w[:, None] * np.cos(ang * np.outer(jm, a))  # [128, 256]
    sw2 = -s * w[:, None] * np.sin(ang * np.outer(jm, a))

    cblob = np.zeros((N, CBLOB_COLS), dtype=bf)
    cblob[:, 0:256] = Co.astype(bf)
    cblob[:, 256:512] = Si.astype(bf)
    cblob[0:128, 512:768] = cw2.astype(bf)
    cblob[0:128, 768:1024] = sw2.astype(bf)

    return dict(cosih=cosih, cblob=np.ascontiguousarray(cblob))


def build_program(n_imgs=IMGS_PER_CORE, n_cores=N_CORES):
    """Build the Bass/Tile program. Returns nc."""
    from contextlib import ExitStack

    import concourse.bacc as bacc
    import concourse.tile as tile
    from concourse import mybir

    f32 = mybir.dt.float32
    f32r = mybir.dt.float32r
    bf16 = mybir.dt.bfloat16

    S = 1.0 / float(N) ** 4  # j=0 y-weight (constant over y)

    nc = bacc.Bacc(
        "TRN2",
        target_bir_lowering=False,
        debug=False,
        num_devices=n_cores,
    )

    x_d = nc.dram_tensor("x", [n_imgs, N, N], f32, kind="ExternalInput").ap()
    cosih_d = nc.dram_tensor("cosih", [N, 258], f32, kind="ExternalInput").ap()
    cblob_d = nc.dram_tensor(
        "cblob", [N, CBLOB_COLS], bf16, kind="ExternalInput"
    ).ap()
    out_d = nc.dram_tensor("out", [n_imgs, N, N], f32, kind="ExternalOutput").ap()

    with tile.TileContext(nc) as tc, ExitStack() as ctx:
        singles = ctx.enter_context(tc.tile_pool(name="singles", bufs=1))
        dpool = ctx.enter_context(tc.tile_pool(name="dpool", bufs=4))
        uvpool = ctx.enter_context(tc.tile_pool(name="uvpool", bufs=4))
        ppool = ctx.enter_context(tc.tile_pool(name="ppool", bufs=4))
        tpool = ctx.enter_context(tc.tile_pool(name="tpool", bufs=6))
        qpool = ctx.enter_context(tc.tile_pool(name="qpool", bufs=3))
        opool = ctx.enter_context(tc.tile_pool(name="opool", bufs=6))
        ps1 = ctx.enter_context(tc.tile_pool(name="ps1", bufs=1, space="PSUM"))
        ps2 = ctx.enter_context(tc.tile_pool(name="ps2", bufs=1, space="PSUM"))
        ps4 = ctx.enter_context(tc.tile_pool(name="ps4", bufs=1, space="PSUM"))
        ps4b = ctx.enter_context(tc.tile_pool(name="ps4b", bufs=1, space="PSUM"))
        ps5 = ctx.enter_context(tc.tile_pool(name="ps5", bufs=1, space="PSUM"))

        # --- constants into SBUF ---
        cosih = [singles.tile([128, 258], f32r, tag=f"cosih{h}", name=f"cosih{h}") for h in range(2)]
        cosib = [singles.tile([128, 512], bf16, tag=f"cosib{h}", name=f"cosib{h}") for h in range(2)]
        for h in range(2):
            sl = slice(128 * h, 128 * (h + 1))
            nc.gpsimd.dma_start(out=cosih[h], in_=cosih_d[sl, :].bitcast(f32r))
            nc.gpsimd.dma_start(out=cosib[h], in_=cblob_d[sl, 0:512])
        cwsw = singles.tile([128, 512], bf16, tag="cwsw", name="cwsw")
        nc.gpsimd.dma_start(out=cwsw, in_=cblob_d[0:128, 512:1024])
        cw2 = cwsw[:, 0:256]
        sw2 = cwsw[:, 256:512]
        # j=0 inverse y-weight is the constant S for every y: memset, no DMA
        cwz = singles.tile([1, 256], bf16, tag="cwz", name="cwz")
        nc.vector.memset(cwz, S)

        mm = nc.tensor.matmul

        st = {}

        def loadA(i):
            # prefetch the image two pipeline slots ahead of its s1 matmuls
            d = dpool.tile([128, 2, N], f32r, tag="d", name="d")
            nc.sync.dma_start(
                out=d,
                in_=x_d[i].rearrange("(h p) c -> p h c", h=2).bitcast(f32r),
            )
            st[i] = {"d": d}

        def compA(i):
            # s1: [U|V] = d^T [CoH|SiH]  (fp32r, FD=258)
            d = st[i]["d"]
            p1 = ps1.tile([128, 2, 512], f32, tag="s1", name="s1")
            for xh in range(2):
                xs = slice(128 * xh, 128 * (xh + 1))
                mm(p1[:, xh, 0:258], d[:, 0, xs], cosih[0], start=True, stop=False)
                mm(p1[:, xh, 0:258], d[:, 1, xs], cosih[1], start=False, stop=True)
            uv = uvpool.tile([128, 2, 258], bf16, tag="uv", name="uv")
            nc.vector.tensor_copy(out=uv, in_=p1[:, :, 0:258])
            # swapped operand [-V|U] built on the (otherwise idle) Pool engine
            uvs = uvpool.tile([128, 2, 258], bf16, tag="uvs", name="uvs")
            nc.gpsimd.tensor_scalar_mul(uvs[:, :, 0:129], uv[:, :, 129:258], -1.0)
            nc.gpsimd.tensor_copy(out=uvs[:, :, 129:258], in_=uv[:, :, 0:129])
            st[i]["uv"] = uv
            st[i]["uvs"] = uvs

        def stageB(i):
            # s2: G1 = Co U - Si V ; G2 = Si U + Co V  (bf16, FD=258):
            # Co x [U|V] writes [CoU|CoV]; Si x [-V|U] adds [-SiV|SiU]
            # -> [G1|G2] in one accumulation group per k-chunk.
            # then P = G1^2 + G2^2 with DC bin zeroed
            uv = st[i]["uv"]
            uvs = st[i]["uvs"]
            p2 = ps2.tile([128, 2, 512], f32, tag="s2", name="s2")
            for kt in range(2):
                ks = slice(128 * kt, 128 * (kt + 1))
                ss = slice(256 + 128 * kt, 256 + 128 * (kt + 1))
                out = p2[:, kt, 0:258]
                mm(out, cosib[0][:, ks], uv[:, 0, :], start=True, stop=False)
                mm(out, cosib[1][:, ks], uv[:, 1, :], start=False, stop=False)
                mm(out, cosib[0][:, ss], uvs[:, 0, :], start=False, stop=False)
                mm(out, cosib[1][:, ss], uvs[:, 1, :], start=False, stop=True)
            t1 = tpool.tile([128, 2, 129], bf16, tag="t1", name="t1")
            nc.scalar.activation(out=t1, in_=p2[:, :, 0:129],
                                 func=mybir.ActivationFunctionType.Square)
            t2 = tpool.tile([128, 2, 129], bf16, tag="t2", name="t2")
            nc.scalar.activation(out=t2, in_=p2[:, :, 129:258],
                                 func=mybir.ActivationFunctionType.Square)
            P = ppool.tile([128, 2, 129], bf16, tag="P", name="P")
            nc.vector.tensor_add(P, t1, t2)
            nc.vector.memset(P[0:1, 0, 0:1], 0.0)
            st[i]["P"] = P

        pairst = {}

        def stageC(i):
            # s4a: Qr|Qi for j=1..128 (dense block) + q0 = P[:,0]^T Co.
            # Results for image pairs (2k, 2k+1) land in shared pair tiles so
            # stageD can run one matmul per weight for BOTH images (s4b
            # weights are constants -> half the instruction count).
            P = st[i]["P"]
            par = i % 2
            if par == 0:
                pairst[i // 2] = {
                    "qp": qpool.tile([128, 2, 512], bf16, tag="qp", name="qp"),
                    "p4r": ps4b.tile([1, 512], f32, tag="s4a0", name="s4a0"),
                }
            pp = pairst[i // 2]
            p4 = ps4.tile([128, 512], f32, tag="s4a", name="s4a")
            mm(p4, P[:, 0, 1:129], cosib[0], start=True, stop=False)
            mm(p4, P[:, 1, 1:129], cosib[1], start=False, stop=True)
            p4r = pp["p4r"][:, 256 * par:256 * (par + 1)]
            mm(p4r, P[:, 0, 0:1], cosib[0][:, 0:256], start=True, stop=False)
            mm(p4r, P[:, 1, 0:1], cosib[1][:, 0:256], start=False, stop=True)
            if par == 1:
                # q0p copy first: it releases the single ps4b buffer, which
                # the next pair's rank-1 matmuls wait on
                q0p = qpool.tile([1, 512], bf16, tag="q0p", name="q0p")
                nc.scalar.activation(out=q0p, in_=pp["p4r"],
                                     func=mybir.ActivationFunctionType.Copy)
                pp["q0p"] = q0p
            nc.scalar.activation(out=pp["qp"][:, par, :], in_=p4,
                                 func=mybir.ActivationFunctionType.Copy)

        def stageD(i):
            # s4b for the pair (i-1, i), i odd. Moving dim spans both
            # images: out rows y-half, cols [imgA x | imgB x].
            pp = pairst[i // 2]
            qp = pp["qp"]
            q0p = pp["q0p"]
            p5 = ps5.tile([128, 2, 512], f32, tag="s4b", name="s4b")
            for h in range(2):
                ys = slice(128 * h, 128 * (h + 1))
                outg = p5[:, h, :]
                mm(outg, cw2[:, ys], qp[:, :, 0:256], start=True, stop=False)
                mm(outg, cwz[:, ys], q0p, start=False, stop=False)
                mm(outg, sw2[:, ys], qp[:, :, 256:512], start=False, stop=True)
            o = opool.tile([128, 2, 512], f32, tag="o", name="o")
            nc.vector.tensor_copy(out=o, in_=p5)
            # issue out-DMAs from the Pool queue: the Act queue must stay
            # clear to run the squares that release the single ps2 buffer
            for par in range(2):
                nc.gpsimd.dma_start(
                    out=out_d[i - 1 + par].rearrange("(h p) c -> p h c", h=2),
                    in_=o[:, :, 256 * par:256 * (par + 1)],
                )
            del st[i - 1]
            del st[i]
            del pairst[i // 2]

        # software pipeline: interleave images, deepest stage first, so no
        # engine's instruction stream blocks on a same-image downstream dep.
        # Input DMA is prefetched 2 slots ahead; stageD fires once per pair.
        for t in range(n_imgs + 6):
            if t < n_imgs:
                loadA(t)
            if 0 <= t - 2 < n_imgs:
                compA(t - 2)
            if 0 <= t - 6 < n_imgs and (t - 6) % 2 == 1:
                stageD(t - 6)
            if 0 <= t - 4 < n_imgs:
                stageC(t - 4)
            if 0 <= t - 3 < n_imgs:
                stageB(t - 3)

    nc.compile()
    return nc


_CACHED = {}


def _get_program(n_imgs, n_cores):
    key = (n_imgs, n_cores)
    if key not in _CACHED:
        _CACHED[key] = build_program(n_imgs, n_cores)
    return _CACHED[key]


def _make_runner(nc, n_cores):
    """Compile a reusable fast-dispatch jitted runner for nc."""
    import jax
    from jax.sharding import Mesh, PartitionSpec, NamedSharding
    from jax.experimental.shard_map import shard_map
    from concourse import mybir
    from concourse.bass2jax import (
        _bass_exec_p,
        install_neuronx_cc_hook,
        partition_id_tensor,
        fast_dispatch_compile,
    )

    install_neuronx_cc_hook()

    partition_name = nc.partition_id_tensor.name if nc.partition_id_tensor else None
    in_names, out_names, out_avals, zero_outs = [], [], [], []
    for alloc in nc.m.functions[0].allocations:
        if not isinstance(alloc, mybir.MemoryLocationSet):
            continue
        name = alloc.memorylocations[0].name
        if alloc.kind == "ExternalInput":
            if name != partition_name:
                in_names.append(name)
        elif alloc.kind == "ExternalOutput":
            shape = tuple(alloc.tensor_shape)
            dtype = mybir.dt.np(alloc.dtype)
            out_names.append(name)
            out_avals.append(jax.core.ShapedArray(shape, dtype))
            zero_outs.append(np.zeros(shape, dtype))
    all_in = in_names + out_names
    if partition_name is not None:
        all_in = all_in + [partition_name]

    def _body(*args):
        operands = list(args)
        if partition_name is not None:
            operands.append(partition_id_tensor())
        outs = _bass_exec_p.bind(
            *operands,
            out_avals=tuple(out_avals),
            in_names=tuple(all_in),
            out_names=tuple(out_names),
            lowering_input_output_aliases=(),
            sim_require_finite=True,
            sim_require_nnan=True,
            nc=nc,
        )
        return tuple(outs)

    devices = jax.devices()[:n_cores]
    mesh = Mesh(np.asarray(devices), ("core",))
    nsh = NamedSharding(mesh, PartitionSpec("core"))
    in_specs = (PartitionSpec("core"),) * (len(in_names) + len(out_names))
    out_specs = (PartitionSpec("core"),) * len(out_names)
    jfn = jax.jit(
        shard_map(_body, mesh=mesh, in_specs=in_specs, out_specs=out_specs,
                  check_rep=False),
        keep_unused=True,
    )

    class Runner:
        pass

    r = Runner()
    r.in_names = in_names
    r.out_names = out_names
    r.zero_outs = zero_outs
    r.nsh = nsh
    r.jfn = jfn
    r.compiled = None

    def run(in_maps):
        concat = [
            np.concatenate([np.asarray(in_maps[c][k]) for c in range(n_cores)], 0)
            for k in in_names
        ]
        concat += [np.concatenate([z] * n_cores, 0) for z in zero_outs]
        dev = [jax.device_put(a, nsh) for a in concat]
        if r.compiled is None:
            r.compiled = fast_dispatch_compile(lambda: jfn.lower(*dev).compile())
        outs = r.compiled(*dev)
        jax.block_until_ready(outs)
        np_outs = [np.asarray(o) for o in outs]
        return [
            {k: np_outs[i].reshape(n_cores, -1, *np_outs[i].shape[1:])[c]
             for i, k in enumerate(out_names)}
            for c in range(n_cores)
        ]

    r.run = run
    return r


_RUNNER = {}


def _get_runner(n_imgs, n_cores):
    key = (n_imgs, n_cores)
    if key not in _RUNNER:
        _RUNNER[key] = _make_runner(_get_program(n_imgs, n_cores), n_cores)
    return _RUNNER[key]


def _shard_inputs(inputs):
    consts = _make_consts()
    bpc = B // N_CORES
    in_maps = []
    for core in range(N_CORES):
        shard = inputs[core * bpc:(core + 1) * bpc]  # [8, 256, 256, 8]
        shard = np.ascontiguousarray(shard.transpose(0, 3, 1, 2)).reshape(
            IMGS_PER_CORE, H, W
        )
        m = {"x": shard}
        m.update(consts)
        in_maps.append(m)
    return in_maps


def kernel(inputs: np.ndarray) -> np.ndarray:
    """inputs: [64, 256, 256, 8] float32 -> output same shape."""
    inputs = np.asarray(inputs, dtype=np.float32)
    assert inputs.shape == (B, H, W, C)

    runner = _get_runner(IMGS_PER_CORE, N_CORES)
    per_core = runner.run(_shard_inputs(inputs))

    bpc = B // N_CORES
    out = np.empty((B, H, W, C), dtype=np.float32)
    for core in range(N_CORES):
        o = per_core[core]["out"].reshape(bpc, C, H, W)
        out[core * bpc:(core + 1) * bpc] = o.transpose(0, 2, 3, 1)
    return out


if __name__ == "__main__":
    rng = np.random.default_rng(0)
    x = rng.standard_normal((B, H, W, C)).astype(np.float32)
    y = kernel(x)
    print("kernel output:", y.shape, y.dtype)


# revision 32
# speedup vs baseline: 1.1353x; 1.1353x over previous
"""Trainium2 Bass kernel for per-(batch,channel) circular autocorrelation:

    out = ifft2(|fft2(img - mean(img))|^2).real / (H*W)

Implemented as dense DFT matmuls on the TensorEngine (no FFT primitive on
TRN2). Key algebra:
  * mean subtraction == zeroing the DC bin of the power spectrum, so we
    transform the raw image and memset P[0,0] = 0.
  * real input => half spectrum (j = 0..128) suffices; mirror rows are
    folded into the inverse-transform constants with weight 2.
  * DFT matrix C = cos - i sin is symmetric, which lets every stage run as
    out = lhsT.T @ rhs with operands already in natural layout (no
    transposes anywhere).
  * the 129-row half spectrum is split as j=1..128 (dense 128-row block)
    plus a j=0 rank-1 fixup. The j=0 row only needs the cosine half of the
    inverse transform (sin(0)=0) and its y-weight is the constant 1/N^4,
    so the fixup costs two narrow matmuls.

Stages per 256x256 image (Co/Si are 256x256 cos/sin matrices):
  s1: [U|V] = d^T [CoH|SiH]              (fp32r matmuls, FD=258)
  s2: [G2|-G1] accumulated in PSUM: wide Si x [U|V] plus narrow +Co x V /
      -Co x U; the -G1 sign is absorbed by the square  (bf16)
  P:  Pt = G1^2 + G2^2 (bf16), Pt[0,0]=0
  s4a: Qr|Qi = Pt[:,1:129]^T [Co|Si] ; q0 = Pt[:,0]^T Co   (bf16)
  s4b: out[y,x] = sum_{j=1..128} w_j/N^4 (cos(2pi jy/N) Qr - sin(..) Qi)
       + q0[x]/N^4   for all 256 y rows (mirror weights baked in).

Sharding: pure data parallel over 8 cores, 64 images (8 batches x 8
channels) per core. All bf16 constants ship as ONE dram tensor (fewer
runtime buffers per call = lower per-call dispatch cost).
"""

import numpy as np

N = 256
J = N // 2 + 1  # 129
B, H, W, C = 64, 256, 256, 8
N_CORES = 8
IMGS_PER_CORE = (B // N_CORES) * C  # 64

# bf16 const blob layout: [256, 1540] (one runtime buffer for ALL constants;
# fewer per-call buffers = lower dispatch cost)
#   cols    0:512  rows 0:256 -> [Co | Si]
#   cols 512:1024  rows 0:128 -> [cw2 | sw2]    (j = 1..128 inverse weights)
#   cols 1024:1540 rows 0:256 -> raw f32 [CoH | SiH] (258 f32 bitcast as bf16)
CBLOB_COLS = 1540


def _make_consts():
    import ml_dtypes

    bf = ml_dtypes.bfloat16
    a = np.arange(N, dtype=np.float64)
    j = np.arange(J, dtype=np.float64)
    ang = 2.0 * np.pi / N

    CoH = np.cos(ang * np.outer(a, j))  # [256, 129]
    SiH = np.sin(ang * np.outer(a, j))
    cosih = np.concatenate([CoH, SiH], axis=1).astype(np.float32)  # [256, 258]

    Co = np.cos(ang * np.outer(a, a))
    Si = np.sin(ang * np.outer(a, a))

    s = 1.0 / float(N) ** 4
    jm = np.arange(1, 129, dtype=np.float64)  # j = 1..128
    w = np.full(128, 2.0)
    w[127] = 1.0  # j=128 self-conjugate row
    cw2 = s * w[:, None] * np.cos(ang * np.outer(jm, a))  # [128, 256]
    sw2 = -s * w[:, None] * np.sin(ang * np.outer(jm, a))

    cblob = np.zeros((N, CBLOB_COLS), dtype=bf)
    cblob[:, 0:256] = Co.astype(bf)
    cblob[:, 256:512] = Si.astype(bf)
    cblob[0:128, 512:768] = cw2.astype(bf)
    cblob[0:128, 768:1024] = sw2.astype(bf)
    cblob[:, 1024:1540] = cosih.view(np.uint16).view(bf)

    return dict(cblob=np.ascontiguousarray(cblob))


def build_program(n_imgs=IMGS_PER_CORE, n_cores=N_CORES):
    """Build the Bass/Tile program. Returns nc."""
    from contextlib import ExitStack

    import concourse.bacc as bacc
    import concourse.tile as tile
    from concourse import mybir

    f32 = mybir.dt.float32
    f32r = mybir.dt.float32r
    bf16 = mybir.dt.bfloat16

    S = 1.0 / float(N) ** 4  # j=0 y-weight (constant over y)

    nc = bacc.Bacc(
        "TRN2",
        target_bir_lowering=False,
        debug=False,
        num_devices=n_cores,
    )

    x_d = nc.dram_tensor("x", [n_imgs, N, N], f32, kind="ExternalInput").ap()
    cblob_d = nc.dram_tensor(
        "cblob", [N, CBLOB_COLS], bf16, kind="ExternalInput"
    ).ap()
    out_d = nc.dram_tensor("out", [n_imgs, N, N], f32, kind="ExternalOutput").ap()

    with tile.TileContext(nc) as tc, ExitStack() as ctx:
        singles = ctx.enter_context(tc.tile_pool(name="singles", bufs=1))
        dpool = ctx.enter_context(tc.tile_pool(name="dpool", bufs=4))
        uvpool = ctx.enter_context(tc.tile_pool(name="uvpool", bufs=4))
        ppool = ctx.enter_context(tc.tile_pool(name="ppool", bufs=4))
        tpool = ctx.enter_context(tc.tile_pool(name="tpool", bufs=6))
        qpool = ctx.enter_context(tc.tile_pool(name="qpool", bufs=3))
        opool = ctx.enter_context(tc.tile_pool(name="opool", bufs=6))
        ps1 = ctx.enter_context(tc.tile_pool(name="ps1", bufs=1, space="PSUM"))
        ps2 = ctx.enter_context(tc.tile_pool(name="ps2", bufs=1, space="PSUM"))
        ps4 = ctx.enter_context(tc.tile_pool(name="ps4", bufs=1, space="PSUM"))
        ps4b = ctx.enter_context(tc.tile_pool(name="ps4b", bufs=1, space="PSUM"))
        ps5 = ctx.enter_context(tc.tile_pool(name="ps5", bufs=1, space="PSUM"))

        # --- constants into SBUF ---
        cosih = [singles.tile([128, 258], f32r, tag=f"cosih{h}", name=f"cosih{h}") for h in range(2)]
        cosib = [singles.tile([128, 512], bf16, tag=f"cosib{h}", name=f"cosib{h}") for h in range(2)]
        for h in range(2):
            sl = slice(128 * h, 128 * (h + 1))
            nc.gpsimd.dma_start(out=cosih[h],
                                in_=cblob_d[sl, 1024:1540].bitcast(f32r))
            nc.gpsimd.dma_start(out=cosib[h], in_=cblob_d[sl, 0:512])
        cwsw = singles.tile([128, 512], bf16, tag="cwsw", name="cwsw")
        nc.gpsimd.dma_start(out=cwsw, in_=cblob_d[0:128, 512:1024])
        cw2 = cwsw[:, 0:256]
        sw2 = cwsw[:, 256:512]
        # j=0 inverse y-weight is the constant S for every y: memset, no DMA
        cwz = singles.tile([1, 256], bf16, tag="cwz", name="cwz")
        nc.vector.memset(cwz, S)

        mm = nc.tensor.matmul

        st = {}

        def loadA(i):
            # prefetch the image two pipeline slots ahead of its s1 matmuls
            d = dpool.tile([128, 2, N], f32r, tag="d", name="d")
            nc.sync.dma_start(
                out=d,
                in_=x_d[i].rearrange("(h p) c -> p h c", h=2).bitcast(f32r),
            )
            st[i] = {"d": d}

        def compA(i):
            # s1: [U|V] = d^T [CoH|SiH]  (fp32r, FD=258)
            d = st[i]["d"]
            p1 = ps1.tile([128, 2, 512], f32, tag="s1", name="s1")
            for xh in range(2):
                xs = slice(128 * xh, 128 * (xh + 1))
                mm(p1[:, xh, 0:258], d[:, 0, xs], cosih[0], start=True, stop=False)
                mm(p1[:, xh, 0:258], d[:, 1, xs], cosih[1], start=False, stop=True)
            uv = uvpool.tile([128, 2, 258], bf16, tag="uv", name="uv")
            nc.vector.tensor_copy(out=uv, in_=p1[:, :, 0:258])
            # swapped operand [-V|U] built on the (otherwise idle) Pool engine
            uvs = uvpool.tile([128, 2, 258], bf16, tag="uvs", name="uvs")
            nc.gpsimd.tensor_scalar_mul(uvs[:, :, 0:129], uv[:, :, 129:258], -1.0)
            nc.gpsimd.tensor_copy(out=uvs[:, :, 129:258], in_=uv[:, :, 0:129])
            st[i]["uv"] = uv
            st[i]["uvs"] = uvs

        def stageB(i):
            # s2: G1 = Co U - Si V ; G2 = Si U + Co V  (bf16, FD=258):
            # Co x [U|V] writes [CoU|CoV]; Si x [-V|U] adds [-SiV|SiU]
            # -> [G1|G2] in one accumulation group per k-chunk.
            # then P = G1^2 + G2^2 with DC bin zeroed
            uv = st[i]["uv"]
            uvs = st[i]["uvs"]
            p2 = ps2.tile([128, 2, 512], f32, tag="s2", name="s2")
            for kt in range(2):
                ks = slice(128 * kt, 128 * (kt + 1))
                ss = slice(256 + 128 * kt, 256 + 128 * (kt + 1))
                out = p2[:, kt, 0:258]
                mm(out, cosib[0][:, ks], uv[:, 0, :], start=True, stop=False)
                mm(out, cosib[1][:, ks], uv[:, 1, :], start=False, stop=False)
                mm(out, cosib[0][:, ss], uvs[:, 0, :], start=False, stop=False)
                mm(out, cosib[1][:, ss], uvs[:, 1, :], start=False, stop=True)
            t1 = tpool.tile([128, 2, 129], bf16, tag="t1", name="t1")
            nc.scalar.activation(out=t1, in_=p2[:, :, 0:129],
                                 func=mybir.ActivationFunctionType.Square)
            t2 = tpool.tile([128, 2, 129], bf16, tag="t2", name="t2")
            nc.scalar.activation(out=t2, in_=p2[:, :, 129:258],
                                 func=mybir.ActivationFunctionType.Square)
            P = ppool.tile([128, 2, 129], bf16, tag="P", name="P")
            nc.vector.tensor_add(P, t1, t2)
            nc.vector.memset(P[0:1, 0, 0:1], 0.0)
            st[i]["P"] = P

        pairst = {}

        def stageC(i):
            # s4a: Qr|Qi for j=1..128 (dense block) + q0 = P[:,0]^T Co.
            # Results for image pairs (2k, 2k+1) land in shared pair tiles so
            # stageD can run one matmul per weight for BOTH images (s4b
            # weights are constants -> half the instruction count).
            P = st[i]["P"]
            par = i % 2
            if par == 0:
                pairst[i // 2] = {
                    "qp": qpool.tile([128, 2, 512], bf16, tag="qp", name="qp"),
                    "p4r": ps4b.tile([1, 512], f32, tag="s4a0", name="s4a0"),
                }
            pp = pairst[i // 2]
            p4 = ps4.tile([128, 512], f32, tag="s4a", name="s4a")
            mm(p4, P[:, 0, 1:129], cosib[0], start=True, stop=False)
            mm(p4, P[:, 1, 1:129], cosib[1], start=False, stop=True)
            p4r = pp["p4r"][:, 256 * par:256 * (par + 1)]
            mm(p4r, P[:, 0, 0:1], cosib[0][:, 0:256], start=True, stop=False)
            mm(p4r, P[:, 1, 0:1], cosib[1][:, 0:256], start=False, stop=True)
            if par == 1:
                # q0p copy first: it releases the single ps4b buffer, which
                # the next pair's rank-1 matmuls wait on
                q0p = qpool.tile([1, 512], bf16, tag="q0p", name="q0p")
                nc.scalar.activation(out=q0p, in_=pp["p4r"],
                                     func=mybir.ActivationFunctionType.Copy)
                pp["q0p"] = q0p
            nc.scalar.activation(out=pp["qp"][:, par, :], in_=p4,
                                 func=mybir.ActivationFunctionType.Copy)

        def stageD(i):
            # s4b for the pair (i-1, i), i odd. Moving dim spans both
            # images: out rows y-half, cols [imgA x | imgB x].
            pp = pairst[i // 2]
            qp = pp["qp"]
            q0p = pp["q0p"]
            p5 = ps5.tile([128, 2, 512], f32, tag="s4b", name="s4b")
            for h in range(2):
                ys = slice(128 * h, 128 * (h + 1))
                outg = p5[:, h, :]
                mm(outg, cw2[:, ys], qp[:, :, 0:256], start=True, stop=False)
                mm(outg, cwz[:, ys], q0p, start=False, stop=False)
                mm(outg, sw2[:, ys], qp[:, :, 256:512], start=False, stop=True)
            o = opool.tile([128, 2, 512], f32, tag="o", name="o")
            nc.vector.tensor_copy(out=o, in_=p5)
            # issue out-DMAs from the Pool queue: the Act queue must stay
            # clear to run the squares that release the single ps2 buffer
            for par in range(2):
                nc.gpsimd.dma_start(
                    out=out_d[i - 1 + par].rearrange("(h p) c -> p h c", h=2),
                    in_=o[:, :, 256 * par:256 * (par + 1)],
                )
            del st[i - 1]
            del st[i]
            del pairst[i // 2]

        # software pipeline: interleave images, deepest stage first, so no
        # engine's instruction stream blocks on a same-image downstream dep.
        # Input DMA is prefetched 2 slots ahead; stageD fires once per pair.
        for t in range(n_imgs + 6):
            if t < n_imgs:
                loadA(t)
            if 0 <= t - 2 < n_imgs:
                compA(t - 2)
            if 0 <= t - 6 < n_imgs and (t - 6) % 2 == 1:
                stageD(t - 6)
            if 0 <= t - 4 < n_imgs:
                stageC(t - 4)
            if 0 <= t - 3 < n_imgs:
                stageB(t - 3)

    nc.compile()
    return nc


_CACHED = {}


def _get_program(n_imgs, n_cores):
    key = (n_imgs, n_cores)
    if key not in _CACHED:
        _CACHED[key] = build_program(n_imgs, n_cores)
    return _CACHED[key]


def _make_runner(nc, n_cores):
    """Compile a reusable fast-dispatch jitted runner for nc."""
    import jax
    from jax.sharding import Mesh, PartitionSpec, NamedSharding
    from jax.experimental.shard_map import shard_map
    from concourse import mybir
    from concourse.bass2jax import (
        _bass_exec_p,
        install_neuronx_cc_hook,
        partition_id_tensor,
        fast_dispatch_compile,
    )

    install_neuronx_cc_hook()

    partition_name = nc.partition_id_tensor.name if nc.partition_id_tensor else None
    in_names, out_names, out_avals, zero_outs = [], [], [], []
    for alloc in nc.m.functions[0].allocations:
        if not isinstance(alloc, mybir.MemoryLocationSet):
            continue
        name = alloc.memorylocations[0].name
        if alloc.kind == "ExternalInput":
            if name != partition_name:
                in_names.append(name)
        elif alloc.kind == "ExternalOutput":
            shape = tuple(alloc.tensor_shape)
            dtype = mybir.dt.np(alloc.dtype)
            out_names.append(name)
            out_avals.append(jax.core.ShapedArray(shape, dtype))
            zero_outs.append(np.zeros(shape, dtype))
    all_in = in_names + out_names
    if partition_name is not None:
        all_in = all_in + [partition_name]

    def _body(*args):
        operands = list(args)
        if partition_name is not None:
            operands.append(partition_id_tensor())
        outs = _bass_exec_p.bind(
            *operands,
            out_avals=tuple(out_avals),
            in_names=tuple(all_in),
            out_names=tuple(out_names),
            lowering_input_output_aliases=(),
            sim_require_finite=True,
            sim_require_nnan=True,
            nc=nc,
        )
        return tuple(outs)

    devices = jax.devices()[:n_cores]
    mesh = Mesh(np.asarray(devices), ("core",))
    nsh = NamedSharding(mesh, PartitionSpec("core"))
    in_specs = (PartitionSpec("core"),) * (len(in_names) + len(out_names))
    out_specs = (PartitionSpec("core"),) * len(out_names)
    jfn = jax.jit(
        shard_map(_body, mesh=mesh, in_specs=in_specs, out_specs=out_specs,
                  check_rep=False),
        keep_unused=True,
    )

    class Runner:
        pass

    r = Runner()
    r.in_names = in_names
    r.out_names = out_names
    r.zero_outs = zero_outs
    r.nsh = nsh
    r.jfn = jfn
    r.compiled = None

    def run(in_maps):
        concat = [
            np.concatenate([np.asarray(in_maps[c][k]) for c in range(n_cores)], 0)
            for k in in_names
        ]
        concat += [np.concatenate([z] * n_cores, 0) for z in zero_outs]
        dev = [jax.device_put(a, nsh) for a in concat]
        if r.compiled is None:
            r.compiled = fast_dispatch_compile(lambda: jfn.lower(*dev).compile())
        outs = r.compiled(*dev)
        jax.block_until_ready(outs)
        np_outs = [np.asarray(o) for o in outs]
        return [
            {k: np_outs[i].reshape(n_cores, -1, *np_outs[i].shape[1:])[c]
             for i, k in enumerate(out_names)}
            for c in range(n_cores)
        ]

    r.run = run
    return r


_RUNNER = {}


def _get_runner(n_imgs, n_cores):
    key = (n_imgs, n_cores)
    if key not in _RUNNER:
        _RUNNER[key] = _make_runner(_get_program(n_imgs, n_cores), n_cores)
    return _RUNNER[key]


def _shard_inputs(inputs):
    consts = _make_consts()
    bpc = B // N_CORES
    in_maps = []
    for core in range(N_CORES):
        shard = inputs[core * bpc:(core + 1) * bpc]  # [8, 256, 256, 8]
        shard = np.ascontiguousarray(shard.transpose(0, 3, 1, 2)).reshape(
            IMGS_PER_CORE, H, W
        )
        m = {"x": shard}
        m.update(consts)
        in_maps.append(m)
    return in_maps


def kernel(inputs: np.ndarray) -> np.ndarray:
    """inputs: [64, 256, 256, 8] float32 -> output same shape."""
    inputs = np.asarray(inputs, dtype=np.float32)
    assert inputs.shape == (B, H, W, C)

    runner = _get_runner(IMGS_PER_CORE, N_CORES)
    per_core = runner.run(_shard_inputs(inputs))

    bpc = B // N_CORES
    out = np.empty((B, H, W, C), dtype=np.float32)
    for core in range(N_CORES):
        o = per_core[core]["out"].reshape(bpc, C, H, W)
        out[core * bpc:(core + 1) * bpc] = o.transpose(0, 2, 3, 1)
    return out


if __name__ == "__main__":
    rng = np.random.default_rng(0)
    x = rng.standard_normal((B, H, W, C)).astype(np.float32)
    y = kernel(x)
    print("kernel output:", y.shape, y.dtype)
